# revision 2
# baseline (speedup 1.0000x reference)
"""Trainium2 Bass kernel: bilinear grid_sample (align_corners=True).

reference: coord [N,2] in [-1,1], params [1,32,1024,1024] -> out [N,32].

Strategy (8 NeuronCores, data-parallel over queries):
  - Host: build an fp16 "diff quad" table [H*W, 128]: cell (y,x) holds,
    channel-major, (a, dx, dy, dxy) = (v00, v01-v00, v10-v00,
    v11-v10-v01+v00) contiguously (256B).  One gather descriptor per
    query fetches the whole bilinear neighborhood in difference form, so
    the device combine is out = a + fx*dx + fy*dy + (fx*fy)*dxy with
    host-precomputed weights (fx, fy, fx*fy) -- no on-device weight math
    and only 3 of 4 planes need the expand/multiply.
  - Host buckets each core's 250k queries into 32 bands of 32 grid rows
    (band = y0>>5) so the in-band cell index (y0&31)*1024+x0 fits int16
    (dma_gather requirement), sorts in-band by cell for HBM locality,
    pads each band to 8064 slots, and precomputes wrapped int16 index
    tiles plus fp16 (fx, fy, fxfy) triplets per slot.
  - Device: dma_gather descriptor generation is the bottleneck; it runs
    on one Q7 core pair per SWDGE queue, so the 32 band gathers round-
    robin across all 4 queues (4 core pairs generate concurrently,
    ~3.6x).  HWDGE (nc.sync) handles regular loads/stores so the Pool
    engine only runs gather descriptor generation.  Per pair of bands:
    ACT dense-expands the 3 weights over channels (DVE broadcast-operand
    multiplies measured ~7x slower than dense), DVE multiplies + 2
    pair-adds, store fp16.
  - Host de-permutes the padded fp16 outputs back to query order, fp32.
"""

import os
import sys

import numpy as np

for _p in ("/opt/trn_rl_repo",):
    if os.path.isdir(_p) and _p not in sys.path:
        sys.path.insert(0, _p)

from contextlib import ExitStack

import concourse.tile as tile
from concourse import bacc, bass, mybir
from concourse.bass_utils import run_bass_kernel_spmd
from concourse.library_config import mlp

F16 = mybir.dt.float16
F32 = mybir.dt.float32
I16 = mybir.dt.int16

N_POINTS = 2_000_000
C = 32
H = 1024
W = 1024
QUAD = 4 * C  # 128 fp16 elems = 256B per table cell
N_CORES = 8
N_PER_CORE = N_POINTS // N_CORES  # 250_000

BANDS = 32
ROWS_PER_BAND = H // BANDS  # 32
BAND_CELLS = ROWS_PER_BAND * W  # 32768 -> in-band idx fits int16
CAP = 8064  # slots per (core, band); 63*128, observed max 8048 (fixed seed)
GB = 2  # bands per DVE group (amortizes per-instruction overhead)
P = 128
NQ = 4  # SWDGE queues: one Q7 core pair each


def build_program(cap: int, repeat: int = 1):
    assert cap % 128 == 0 and BANDS % GB == 0
    kb = cap // 128
    sb = cap // 16

    nc = bacc.Bacc(
        "TRN2",
        target_bir_lowering=False,
        debug=False,
        num_devices=N_CORES,
        num_swdge_queues=NQ,
    )
    MUL, ADD = mybir.AluOpType.mult, mybir.AluOpType.add
    gkb = GB * kb  # group queries per partition
    G = BANDS // GB

    table_t = nc.dram_tensor("table", [H * W, QUAD], F16, kind="ExternalInput")
    idx_t = nc.dram_tensor("idxs", [G * P, GB * sb], I16, kind="ExternalInput")
    f_t = nc.dram_tensor("fxfy", [G * P, gkb * 3], F16, kind="ExternalInput")
    out_t = nc.dram_tensor("out", [G * P, gkb * C], F16, kind="ExternalOutput")

    with tile.TileContext(nc) as tc, ExitStack() as ctx:
        nc.gpsimd.load_library(mlp)
        in_pool = ctx.enter_context(tc.tile_pool(name="in", bufs=3))
        g_pool = ctx.enter_context(tc.tile_pool(name="g", bufs=3))
        o_pool = ctx.enter_context(tc.tile_pool(name="o", bufs=3))
        wd_pool = ctx.enter_context(tc.tile_pool(name="wd", bufs=2))

        table_ap = table_t.ap()
        idx_ap = idx_t.ap()
        f_ap = f_t.ap()
        out_ap = out_t.ap()

        for _rep in range(repeat):
            for grp in range(G):
                r0 = grp * P

                idx_s = in_pool.tile([P, GB * sb], I16, tag="idx")
                f_s = in_pool.tile([P, gkb * 3], F16, tag="f")
                g = g_pool.tile([P, gkb * QUAD], F16, tag="g")
                g4 = g[:].rearrange("p (k c j) -> p k c j", c=C, j=4)
                g3 = g[:].rearrange("p (k e) -> p k e", e=QUAD)
                nc.sync.dma_start(out=idx_s[:], in_=idx_ap[r0 : r0 + P, :])
                nc.sync.dma_start(out=f_s[:], in_=f_ap[r0 : r0 + P, :])
                for i in range(GB):
                    b = grp * GB + i
                    # bulk gather: band slot q -> g[q%128, i*kb + q//128, :, :]
                    # round-robin queues: each queue = its own Q7 core pair
                    nc.gpsimd.dma_gather(
                        g3[:, i * kb : (i + 1) * kb, :],
                        table_ap[b * BAND_CELLS : (b + 1) * BAND_CELLS, :],
                        idx_s[:, i * sb : (i + 1) * sb],
                        cap,
                        cap,
                        QUAD,
                        single_packet=False,
                        queue_num=b % NQ,
                    )

                # dense-expand (fx, fy, fxfy) over channels on ACT; the DVE
                # multiply then runs all-dense (broadcast operands are ~7x
                # slower on DVE)
                f3 = f_s[:].rearrange("p (k j) -> p k j", j=3)
                w4 = f3.unsqueeze(2).to_broadcast([P, gkb, C, 3])
                wd = wd_pool.tile([P, gkb * C * 3], F16, tag="wd")
                wd4 = wd[:].rearrange("p (k c j) -> p k c j", c=C, j=3)
                nc.scalar.activation(wd4, w4, mybir.ActivationFunctionType.Copy)
                # g[..., 1:4] *= (fx, fy, fxfy); g[..., 0] = a stays
                nc.vector.tensor_tensor(
                    out=g4[:, :, :, 1:4], in0=g4[:, :, :, 1:4], in1=wd4, op=MUL
                )
                # pair add: g[..., 0:2] += g[..., 2:4]
                nc.vector.tensor_tensor(
                    out=g4[:, :, :, 0:2],
                    in0=g4[:, :, :, 0:2],
                    in1=g4[:, :, :, 2:4],
                    op=ADD,
                )
                o = o_pool.tile([P, gkb * C], F16, tag="o")
                o3 = o[:].rearrange("p (k c) -> p k c", c=C)
                nc.vector.tensor_tensor(
                    out=o3,
                    in0=g4[:, :, :, 0:1].squeeze(3),
                    in1=g4[:, :, :, 1:2].squeeze(3),
                    op=ADD,
                )

                nc.sync.dma_start(out=out_ap[r0 : r0 + P, :], in_=o[:])

    nc.compile()
    return nc


_nc_cache = {}


def _get_program(cap: int, repeat: int = 1):
    key = (cap, repeat)
    if key not in _nc_cache:
        _nc_cache[key] = build_program(cap, repeat)
    return _nc_cache[key]


def _make_table(params: np.ndarray) -> np.ndarray:
    """fp16 diff-quad table [H*W, 128]: cell = 32 ch x (a, dx, dy, dxy)."""
    v = np.ascontiguousarray(np.transpose(params[0], (1, 2, 0))).astype(np.float32)
    vx = np.concatenate([v[:, 1:], v[:, -1:]], axis=1)
    vy = np.concatenate([v[1:], v[-1:]], axis=0)
    vxy = np.concatenate([vx[1:], vx[-1:]], axis=0)
    a = v
    dx = vx - v
    dy = vy - v
    dxy = vxy - vx - vy + v
    quad = np.stack([a, dx, dy, dxy], axis=-1).astype(np.float16)  # [H,W,C,4]
    return quad.reshape(H * W, QUAD)


def _host_prep(coord: np.ndarray, cap: int):
    """Bucket queries per (core, band); build device input tiles."""
    kb = cap // 128
    sb = cap // 16
    xy = coord.astype(np.float32, copy=False)
    ix = (xy[:, 0] + np.float32(1.0)) * np.float32(0.5) * np.float32(W - 1)
    iy = (xy[:, 1] + np.float32(1.0)) * np.float32(0.5) * np.float32(H - 1)
    x0f = np.floor(ix)
    y0f = np.floor(iy)
    fx32 = ix - x0f
    fy32 = iy - y0f
    fx = fx32.astype(np.float16)
    fy = fy32.astype(np.float16)
    fxy = (fx32 * fy32).astype(np.float16)
    x0 = np.clip(x0f.astype(np.int32), 0, W - 1)
    y0 = np.clip(y0f.astype(np.int32), 0, H - 1)
    band = y0 >> 5
    idx16 = (((y0 & 31) << 10) | x0).astype(np.int16)

    per_core = []
    slotpos_all = []
    for c in range(N_CORES):
        sl = slice(c * N_PER_CORE, (c + 1) * N_PER_CORE)
        bnd = band[sl]
        counts = np.bincount(bnd, minlength=BANDS)
        if counts.max() > cap:
            return None, int(counts.max())  # caller rebuilds with larger cap
        # sort by global cell index: groups by band AND orders in-band
        # accesses by cell for HBM row locality during the gather
        order = np.argsort((y0[sl] << 10) | x0[sl], kind="stable")
        bnd_sorted = bnd[order]
        starts = np.zeros(BANDS, np.int64)
        starts[1:] = np.cumsum(counts)[:-1]
        ranks = np.arange(N_PER_CORE, dtype=np.int64) - starts[bnd_sorted]
        slotpos = np.empty(N_PER_CORE, np.int64)
        slotpos[order] = bnd_sorted * cap + ranks
        slotpos_all.append(slotpos)

        idx_pad = np.zeros(BANDS * cap, np.int16)
        idx_pad[slotpos] = idx16[sl]
        fx_pad = np.zeros(BANDS * cap, np.float16)
        fx_pad[slotpos] = fx[sl]
        fy_pad = np.zeros(BANDS * cap, np.float16)
        fy_pad[slotpos] = fy[sl]
        fxy_pad = np.zeros(BANDS * cap, np.float16)
        fxy_pad[slotpos] = fxy[sl]

        # wrapped int16 idx layout: within a band, i -> [i%16, i//16],
        # replicated 8x across partition groups; bands packed GB per row
        G = BANDS // GB
        iw = np.tile(
            idx_pad.reshape(BANDS, sb, 16).transpose(0, 2, 1), (1, 8, 1)
        )  # [BANDS, 128, sb]
        idx_tile = np.ascontiguousarray(
            iw.reshape(G, GB, P, sb).transpose(0, 2, 1, 3).reshape(G * P, GB * sb)
        )
        # f layout: slot i -> [i%128, (i//128)*3 + {0,1,2}], GB bands/row
        fxs = fx_pad.reshape(BANDS, kb, P).transpose(0, 2, 1)
        fys = fy_pad.reshape(BANDS, kb, P).transpose(0, 2, 1)
        fxys = fxy_pad.reshape(BANDS, kb, P).transpose(0, 2, 1)
        fb = np.stack([fxs, fys, fxys], axis=-1).reshape(BANDS, P, kb * 3)
        f_tile = np.ascontiguousarray(
            fb.reshape(G, GB, P, kb * 3)
            .transpose(0, 2, 1, 3)
            .reshape(G * P, GB * kb * 3)
        )
        per_core.append({"idxs": idx_tile, "fxfy": f_tile})
    return (per_core, slotpos_all), None


def _unshard(results, slotpos_all, cap: int) -> np.ndarray:
    kb = cap // 128
    G = BANDS // GB
    out = np.empty((N_POINTS, C), np.float32)
    for c in range(N_CORES):
        flat = (
            results[c]["out"]
            .reshape(G, P, GB, kb, C)
            .transpose(0, 2, 3, 1, 4)
            .reshape(BANDS * cap, C)
        )
        out[c * N_PER_CORE : (c + 1) * N_PER_CORE] = flat[slotpos_all[c]]
    return out


def _run(coord: np.ndarray, params: np.ndarray, trace: bool = False, **kw):
    assert coord.shape == (N_POINTS, 2) and params.shape == (1, C, H, W)
    cap = CAP
    table = _make_table(params)
    while True:
        prep, max_count = _host_prep(coord, cap)
        if prep is not None:
            break
        cap = ((max_count + 255) // 128) * 128  # rebuild with headroom
    per_core, slotpos_all = prep
    nc = _get_program(cap)
    in_maps = [{"table": table, **per_core[c]} for c in range(N_CORES)]
    res = run_bass_kernel_spmd(nc, in_maps, list(range(N_CORES)), trace=trace, **kw)
    return _unshard(res.results, slotpos_all, cap), res


def kernel(coord: np.ndarray, params: np.ndarray) -> np.ndarray:
    return _run(coord, params)[0]
